# revision 10
# baseline (speedup 1.0000x reference)
"""EquivSetConv (hypergraph message passing) Trainium2 Bass kernel.

Math (reference):
  Xd = segment_sum(dif_vals * X[dif_cols], dif_rows, N)
  Xe = segment_sum((Xd@W1+b1)[vertex], edges, E)
  Xv = segment_sum(concat(Xd[vertex], Xe[edges]) @ W2 + b2, vertex, N)
  out = ((1-a)*Xv + a*Xd) @ W + b

Algebraic reassociation used here (exact up to fp reassociation):
  A[e]  = segment_sum(Xd[vertex], edges, E)
  Xe    = A @ W1 + cnt_e x b1
  B[v]  = segment_sum(Xe[edges], vertex, N)
  Xv    = cnt_v * (Xd @ W2top) + B @ W2bot + cnt_v x b2
  out   = ((1-a)Xv + a Xd) @ W + b

Distribution: nodes sharded 8 ways by row range; incidence lists bucketed by
destination core; the only collective is an AllReduce of the per-core partial
A [E,64]. Segment sums run as one-hot matmul accumulation in PSUM over
128-destination groups; gathers use the MoE dma_gather (int16 idx, <=1024/call).
"""
import sys
import numpy as np

sys.path.insert(0, "/opt/trn_rl_repo")

D = 64
NC = 8
CHUNK = 1024        # dma_gather per-call token cap
MB = 16             # one-hot tiles built per DVE op
ALPHA = 0.5
BUCKET = 32768      # int16 gather index range
TRACE = False
LAST_EXEC_NS = None
LAST_RESULTS = None


def _wrap16(a):
    a = np.asarray(a, np.int16)
    return np.tile(a.reshape(-1, 16).T, (8, 1))  # [128, T/16]


def _wrap128(a):
    return np.ascontiguousarray(np.asarray(a).reshape(-1, 128).T)  # [128, T/128]


def _prep(inputs, n_edges):
    X = np.ascontiguousarray(np.asarray(inputs["X"], np.float32))
    N = X.shape[0]
    assert N % NC == 0
    S = N // NC
    G1 = -(-S // 128)          # node groups per core
    SP = G1 * 128
    EG = -(-n_edges // 128)    # edge groups
    EP = EG * 128
    NB = -(-N // BUCKET)

    dr = np.asarray(inputs["dif_rows"], np.int64)
    dc = np.asarray(inputs["dif_cols"], np.int64)
    dv = np.asarray(inputs["dif_vals"], np.float32)
    vx = np.asarray(inputs["vertex"], np.int64)
    eg = np.asarray(inputs["edges"], np.int64)
    assert eg.max() < n_edges and vx.max() < N and dr.max() < N and dc.max() < N

    # --- global K constants (max cell fill over all cores) ---
    def cellmax(core, cell, ncells):
        cnt = np.bincount(core * ncells + cell, minlength=NC * ncells)
        return int(cnt.max())

    c1 = dr // S
    k1 = -(-cellmax(c1, (dc // BUCKET) * G1 + (dr % S) // 128, NB * G1) // 128)
    c2 = vx // S
    k2 = -(-cellmax(c2, eg // 128, EG) // 128)
    k4 = -(-cellmax(c2, (vx % S) // 128, G1) // 128)
    T1 = NB * G1 * k1 * 128
    T2 = EG * k2 * 128
    T4 = G1 * k4 * 128

    Wf = np.asarray(inputs["W_w"], np.float32)
    W2 = np.asarray(inputs["W2_w"], np.float32)
    W2b = np.asarray(inputs["W2_b"], np.float32)
    Wb = np.asarray(inputs["W_b"], np.float32)
    cnte_g = np.bincount(eg, minlength=EP).astype(np.float32)

    shared = {
        "X": X,
        "W1": np.asarray(inputs["W1_w"], np.float32),
        "W2top": np.ascontiguousarray(W2[:D]),
        "W2bot": np.ascontiguousarray(W2[D:]),
        "Ww1": np.ascontiguousarray((1.0 - ALPHA) * Wf),
        "Ww2": np.ascontiguousarray(ALPHA * Wf),
        "W1b_rep": np.tile(np.asarray(inputs["W1_b"], np.float32), (128, 1)),
        "W2bWw_rep": np.tile((1.0 - ALPHA) * (W2b @ Wf), (128, 1)),
        "Wb_rep": np.tile(Wb, (128, 1)),
        "cnte": _wrap128(cnte_g),
        "iota": np.tile(np.arange(128, dtype=np.float32), (128, 1)),
        "ident": np.eye(128, dtype=np.float32),
    }

    def fill(slots_T, cell_of_tok, kcell, order, gval, dval, vval=None):
        # slots_T: total slots; cell size kcell*128; tokens sorted by `order`.
        cell = cell_of_tok[order]
        g = gval[order]
        d = dval[order]
        if len(cell):
            newc = np.empty(len(cell), bool)
            newc[0] = True
            newc[1:] = cell[1:] != cell[:-1]
            starts = np.where(newc)[0]
            idx = np.arange(len(cell))
            cell_start = np.zeros(len(cell), np.int64)
            cell_start[starts] = idx[starts]
            cell_start = np.maximum.accumulate(cell_start)
            rank = idx - cell_start
        else:
            rank = np.zeros(0, np.int64)
        slot = cell * (kcell * 128) + rank
        assert len(slot) == 0 or rank.max() < kcell * 128
        gi = np.zeros(slots_T, np.int64)
        dl = np.full(slots_T, -1.0, np.float32)
        gi[slot] = g
        dl[slot] = d
        out = [_wrap16(gi), _wrap128(dl)]
        if vval is not None:
            vv = np.zeros(slots_T, np.float32)
            vv[slot] = vval[order]
            out.append(_wrap128(vv))
        return out

    in_maps = []
    for c in range(NC):
        lo = c * S
        m = (dr >= lo) & (dr < lo + S)
        d1 = dr[m] - lo
        c1_, v1 = dc[m], dv[m]
        b1 = c1_ // BUCKET
        cell1 = b1 * G1 + d1 // 128  # bucket-major cell id
        order1 = np.lexsort((d1, b1))
        gi1, dl1, vv1 = fill(T1, cell1, k1, order1, c1_ - b1 * BUCKET, d1 % 128, v1)

        m2 = (vx >= lo) & (vx < lo + S)
        e2, v2 = eg[m2], vx[m2] - lo
        order2 = np.lexsort((e2,))
        gi2, dl2 = fill(T2, e2 // 128, k2, order2, v2, e2 % 128)
        order4 = np.lexsort((v2,))
        gi4, dl4 = fill(T4, v2 // 128, k4, order4, e2, v2 % 128)

        cntv = np.bincount(v2, minlength=SP).astype(np.float32)
        in_maps.append(dict(shared,
                            gidx1=gi1, drel1=dl1, val1=vv1,
                            gidx2=gi2, drel2=dl2,
                            gidx4=gi4, drel4=dl4,
                            cntv=_wrap128(cntv)))

    meta = dict(N=N, S=S, G1=G1, SP=SP, EG=EG, EP=EP, NB=NB,
                K1=k1, K2=k2, K4=k4, T1=T1, T2=T2, T4=T4)
    return meta, in_maps


def _build(meta):
    from concourse import bass, bacc, tile, mybir

    f32, i16 = mybir.dt.float32, mybir.dt.int16
    N, S, G1, SP, EG, EP, NB = (meta[k] for k in
                                ("N", "S", "G1", "SP", "EG", "EP", "NB"))
    K1, K2, K4, T1, T2, T4 = (meta[k] for k in
                              ("K1", "K2", "K4", "T1", "T2", "T4"))

    nc = bacc.Bacc("TRN2", target_bir_lowering=False, debug=False,
                   num_devices=NC)

    def par(name, shape, dt=f32, out=False):
        return nc.declare_dram_parameter(name, list(shape), dt, isOutput=out)

    X = par("X", (N, D))
    gidx1 = par("gidx1", (128, T1 // 16), i16)
    drel1 = par("drel1", (128, T1 // 128))
    val1 = par("val1", (128, T1 // 128))
    gidx2 = par("gidx2", (128, T2 // 16), i16)
    drel2 = par("drel2", (128, T2 // 128))
    gidx4 = par("gidx4", (128, T4 // 16), i16)
    drel4 = par("drel4", (128, T4 // 128))
    cntv = par("cntv", (128, G1))
    cnte = par("cnte", (128, EG))
    W1 = par("W1", (D, D))
    W2top = par("W2top", (D, D))
    W2bot = par("W2bot", (D, D))
    Ww1 = par("Ww1", (D, D))
    Ww2 = par("Ww2", (D, D))
    W1b_rep = par("W1b_rep", (128, D))
    W2bWw_rep = par("W2bWw_rep", (128, D))
    Wb_rep = par("Wb_rep", (128, D))
    iota = par("iota", (128, 128))
    ident = par("ident", (128, 128))
    OUT = par("OUT", (SP, D), out=True)

    eq = mybir.AluOpType.is_equal
    mult = mybir.AluOpType.mult
    addop = mybir.AluOpType.add

    with tile.TileContext(nc) as tc:
        with (
            tc.tile_pool(name="meta1", bufs=1) as metap,
            tc.tile_pool(name="gidxp", bufs=2) as gidxp,
            tc.tile_pool(name="gpool", bufs=6) as gpool,
            tc.tile_pool(name="mpool", bufs=2) as mpool,
            tc.tile_pool(name="psA", bufs=3, space="PSUM") as psA,
            tc.tile_pool(name="psT", bufs=2, space="PSUM") as psT,
            tc.tile_pool(name="small", bufs=3) as small,
            tc.tile_pool(name="stage", bufs=2) as stage,
            tc.tile_pool(name="dram", bufs=1, space="DRAM") as dram,
        ):
            # --- resident metadata ---
            def load(ap_param, shape, nm, dt=f32, pool=metap):
                t = pool.tile(list(shape), dt, name=nm, tag=nm)
                nc.scalar.dma_start(t[:], ap_param[:])
                return t

            iota_t = load(iota, (128, 128), "iota_t")
            ident_t = load(ident, (128, 128), "ident_t")
            w1_t = load(W1, (D, D), "w1_t")
            w2top_t = load(W2top, (D, D), "w2top_t")
            w2bot_t = load(W2bot, (D, D), "w2bot_t")
            ww1_t = load(Ww1, (D, D), "ww1_t")
            ww2_t = load(Ww2, (D, D), "ww2_t")
            w1b_t = load(W1b_rep, (128, D), "w1b_t")
            w2bww_t = load(W2bWw_rep, (128, D), "w2bww_t")
            wb_t = load(Wb_rep, (128, D), "wb_t")
            cntv_t = load(cntv, (128, G1), "cntv_t")
            cnte_t = load(cnte, (128, EG), "cnte_t")
            drel1_t = load(drel1, (128, T1 // 128), "drel1_t")
            val1_t = load(val1, (128, T1 // 128), "val1_t")
            drel2_t = load(drel2, (128, T2 // 128), "drel2_t")
            drel4_t = load(drel4, (128, T4 // 128), "drel4_t")

            Xd_sb = metap.tile([128, G1, D], f32)    # wrapped node shard
            B_sb = metap.tile([128, G1, D], f32)

            Xd_hbm = dram.tile([SP, D], f32)
            Xe_hbm = dram.tile([EP, D], f32)
            A_part = dram.tile([EP, D], f32)
            A_full = dram.tile([EP, D], f32)

            def sparse_step(gidx_par, gidx_cols, drel_t, val_t, srcs,
                            ngrp, kt, evac):
                """srcs: list of (src_ap, slots) bucket streams; total = sum."""
                gidx_t = gidxp.tile([128, gidx_cols], i16, tag="gidx")
                nc.scalar.dma_start(gidx_t[:], gidx_par[:])
                tile_src = {}
                base = 0
                for src_ap, L in srcs:
                    off = 0
                    while off < L:
                        n = min(CHUNK, L - off)
                        cols = n // 128
                        gt = gpool.tile([128, CHUNK // 128, D], f32, tag="g")
                        nc.gpsimd.dma_gather(
                            gt[:, :cols, :], src_ap,
                            gidx_t[:, (base + off) // 16:(base + off + n) // 16],
                            n, n, D)
                        if val_t is not None:
                            vs = val_t[:, (base + off) // 128:
                                       (base + off) // 128 + cols]
                            nc.vector.tensor_mul(
                                gt[:, :cols, :], gt[:, :cols, :],
                                vs.unsqueeze(2).broadcast_to([128, cols, D]))
                        for i in range(cols):
                            tile_src[(base + off) // 128 + i] = (gt, i)
                        off += n
                    base += L
                ntiles = base // 128
                m_buf = None
                cur = None
                for t in range(ntiles):
                    if t % MB == 0:
                        k = min(MB, ntiles - t)
                        m_buf = mpool.tile([128, MB, 128], f32, tag="m")
                        ib = iota_t[:].unsqueeze(1).broadcast_to([128, k, 128])
                        db = drel_t[:, t:t + k].unsqueeze(2).broadcast_to(
                            [128, k, 128])
                        nc.vector.tensor_tensor(m_buf[:, :k, :], ib, db, eq)
                    i = t % kt
                    if i == 0:
                        cur = psA.tile([128, D], f32, tag="acc")
                    gt, col = tile_src[t]
                    nc.tensor.matmul(cur[:], m_buf[:, t % MB, :],
                                     gt[:, col, :],
                                     start=(i == 0), stop=(i == kt - 1))
                    if i == kt - 1:
                        evac(t // kt, cur)

            # ---- step 1: diffusion into Xd ----
            bl1 = G1 * K1 * 128
            srcs1 = []
            for b in range(NB):
                rows = min(BUCKET, N - b * BUCKET)
                srcs1.append((X[b * BUCKET:b * BUCKET + rows, :], bl1))

            def evac1(cellidx, psum):
                b, g = divmod(cellidx, G1)
                if b == 0:
                    nc.vector.tensor_copy(Xd_sb[:, g, :], psum[:])
                else:
                    nc.vector.tensor_add(Xd_sb[:, g, :], Xd_sb[:, g, :],
                                         psum[:])

            sparse_step(gidx1, T1 // 16, drel1_t, val1_t, srcs1, G1, K1, evac1)

            # Xd wrapped -> row-major HBM (gather table for step 2)
            nc.sync.dma_start(
                Xd_hbm[:].rearrange("(g p) f -> p g f", p=128), Xd_sb[:])

            # ---- step 2: A[e] partials ----
            ev2 = {}

            def evac2(g, psum):
                b = g % 4
                if b == 0:
                    ev2["t"] = stage.tile([128, 4, D], f32, tag="ev2", name="ev2t")
                    ev2["g0"] = g
                nc.vector.tensor_copy(ev2["t"][:, b, :], psum[:])
                if b == 3 or g == EG - 1:
                    nb = b + 1
                    nc.sync.dma_start(
                        A_part[ev2["g0"] * 128:(ev2["g0"] + nb) * 128, :]
                        .rearrange("(b p) f -> p b f", p=128),
                        ev2["t"][:, :nb, :])

            sparse_step(gidx2, T2 // 16, drel2_t, None,
                        [(Xd_hbm[:, :], T2)], EG, K2, evac2)

            # ---- AllReduce A ----
            nc.gpsimd.collective_compute(
                "AllReduce", addop,
                replica_groups=[list(range(NC))],
                ins=[A_part.opt()], outs=[A_full.opt()])

            # ---- step 3: Xe = A @ W1 + cnt_e x b1 ----
            for g in range(EG):
                a_t = stage.tile([128, D], f32, tag="a")
                nc.scalar.dma_start(a_t[:], A_full[g * 128:(g + 1) * 128, :])
                pT = psT.tile([D, 128], f32, tag="t")
                nc.tensor.transpose(pT[:], a_t[:], ident_t[:])
                aT = stage.tile([D, 128], f32, tag="aT")
                nc.vector.tensor_copy(aT[:], pT[:])
                p2 = psA.tile([128, D], f32, tag="acc")
                nc.tensor.matmul(p2[:], aT[:], w1_t[:], start=True, stop=True)
                xe = stage.tile([128, D], f32, tag="xe")
                nc.vector.scalar_tensor_tensor(
                    xe[:], w1b_t[:], cnte_t[:, g:g + 1], p2[:], mult, addop)
                nc.sync.dma_start(Xe_hbm[g * 128:(g + 1) * 128, :], xe[:])

            # ---- step 4: B[v] ----
            def evac4(g, psum):
                nc.vector.tensor_copy(B_sb[:, g, :], psum[:])

            sparse_step(gidx4, T4 // 16, drel4_t, None,
                        [(Xe_hbm[:, :], T4)], G1, K4, evac4)

            # ---- steps 5-7 ----
            for g in range(G1):
                xd = Xd_sb[:, g, :]
                cnt = cntv_t[:, g:g + 1]
                xdc = stage.tile([128, D], f32, tag="xdc")
                nc.vector.tensor_scalar_mul(xdc[:], xd, cnt)
                pT1 = psT.tile([D, 128], f32, tag="t")
                nc.tensor.transpose(pT1[:], xdc[:], ident_t[:])
                xdcT = stage.tile([D, 128], f32, tag="xdcT")
                nc.vector.tensor_copy(xdcT[:], pT1[:])
                pT2 = psT.tile([D, 128], f32, tag="t")
                nc.tensor.transpose(pT2[:], xd, ident_t[:])
                xdT = stage.tile([D, 128], f32, tag="xdT")
                nc.vector.tensor_copy(xdT[:], pT2[:])
                pT3 = psT.tile([D, 128], f32, tag="t")
                nc.tensor.transpose(pT3[:], B_sb[:, g, :], ident_t[:])
                bT = stage.tile([D, 128], f32, tag="bT")
                nc.vector.tensor_copy(bT[:], pT3[:])
                pvT = psT.tile([D, 128], f32, tag="vt")
                nc.tensor.matmul(pvT[:], w2top_t[:], xdcT[:],
                                 start=True, stop=False)
                nc.tensor.matmul(pvT[:], w2bot_t[:], bT[:],
                                 start=False, stop=True)
                xvT = stage.tile([D, 128], f32, tag="xvT")
                nc.vector.tensor_copy(xvT[:], pvT[:])
                po = psA.tile([128, D], f32, tag="acc")
                nc.tensor.matmul(po[:], xvT[:], ww1_t[:],
                                 start=True, stop=False)
                nc.tensor.matmul(po[:], xdT[:], ww2_t[:],
                                 start=False, stop=True)
                tmp = stage.tile([128, D], f32, tag="tmp")
                nc.vector.scalar_tensor_tensor(
                    tmp[:], w2bww_t[:], cnt, po[:], mult, addop)
                ot = stage.tile([128, D], f32, tag="ot")
                nc.vector.tensor_add(ot[:], tmp[:], wb_t[:])
                nc.sync.dma_start(OUT[g * 128:(g + 1) * 128, :], ot[:])

    nc.compile()
    return nc


def _run(inputs, n_edges, sim=False):
    meta, in_maps = _prep(inputs, n_edges)
    nc = _build(meta)
    S, SP = meta["S"], meta["SP"]
    if sim:
        from concourse import bass_interp
        ms = bass_interp.MultiCoreSim(nc, NC)
        for c in range(NC):
            for k, v in in_maps[c].items():
                ms.cores[c].tensor(k)[:] = v
        ms.simulate()
        outs = [np.array(ms.cores[c].mem_tensor("OUT")).reshape(SP, D)
                for c in range(NC)]
    else:
        from concourse.bass_utils import run_bass_kernel_spmd
        res = run_bass_kernel_spmd(nc, in_maps, list(range(NC)),
                                   trace=TRACE)
        global LAST_EXEC_NS, LAST_RESULTS
        LAST_EXEC_NS = res.exec_time_ns
        LAST_RESULTS = res
        outs = [res.results[c]["OUT"] for c in range(NC)]
    return np.concatenate([o[:S] for o in outs], axis=0).astype(np.float32)


def kernel(**inputs):
    return _run(inputs, 25000, sim=False)


# revision 13
# speedup vs baseline: 1.9255x; 1.9255x over previous
"""EquivSetConv (hypergraph message passing) Trainium2 Bass kernel.

Math (reference):
  Xd = segment_sum(dif_vals * X[dif_cols], dif_rows, N)
  Xe = segment_sum((Xd@W1+b1)[vertex], edges, E)
  Xv = segment_sum(concat(Xd[vertex], Xe[edges]) @ W2 + b2, vertex, N)
  out = ((1-a)*Xv + a*Xd) @ W + b

Algebraic reassociation used here (exact up to fp reassociation):
  A[e]  = segment_sum(Xd[vertex], edges, E)
  Xe    = A @ W1 + cnt_e x b1
  B[v]  = segment_sum(Xe[edges], vertex, N)
  Xv    = cnt_v * (Xd @ W2top) + B @ W2bot + cnt_v x b2
  out   = ((1-a)Xv + a Xd) @ W + b

Distribution: nodes sharded 8 ways by row range; incidence lists bucketed by
destination core; the only collective is an AllReduce of the per-core partial
A [E,64]. Segment sums run as one-hot matmul accumulation in PSUM over
128-destination groups; gathers use the MoE dma_gather (int16 idx, <=1024/call).
"""
import sys
import numpy as np

sys.path.insert(0, "/opt/trn_rl_repo")

D = 64
NC = 8
CHUNK = 1024        # dma_gather per-call token cap
MB = 16             # one-hot tiles built per DVE op
ALPHA = 0.5
BUCKET = 32768      # int16 gather index range
TRACE = False
LAST_EXEC_NS = None
LAST_RESULTS = None


def _wrap16(a):
    a = np.asarray(a, np.int16)
    return np.tile(a.reshape(-1, 16).T, (8, 1))  # [128, T/16]


def _wrap128(a):
    return np.ascontiguousarray(np.asarray(a).reshape(-1, 128).T)  # [128, T/128]


def _prep(inputs, n_edges):
    X = np.ascontiguousarray(np.asarray(inputs["X"], np.float32))
    N = X.shape[0]
    assert N % NC == 0
    S = N // NC
    G1 = -(-S // 128)          # node groups per core
    SP = G1 * 128
    EG = -(-n_edges // 128)    # edge groups
    EP = EG * 128
    NB = -(-N // BUCKET)

    dr = np.asarray(inputs["dif_rows"], np.int64)
    dc = np.asarray(inputs["dif_cols"], np.int64)
    dv = np.asarray(inputs["dif_vals"], np.float32)
    vx = np.asarray(inputs["vertex"], np.int64)
    eg = np.asarray(inputs["edges"], np.int64)
    assert eg.max() < n_edges and vx.max() < N and dr.max() < N and dc.max() < N

    # --- global K constants (max cell fill over all cores) ---
    def cellmax(core, cell, ncells):
        cnt = np.bincount(core * ncells + cell, minlength=NC * ncells)
        return int(cnt.max())

    c1 = dr // S
    k1 = -(-cellmax(c1, (dc // BUCKET) * G1 + (dr % S) // 128, NB * G1) // 128)
    c2 = vx // S
    k2 = -(-cellmax(c2, eg // 128, EG) // 128)
    k4 = -(-cellmax(c2, (vx % S) // 128, G1) // 128)
    T1 = NB * G1 * k1 * 128
    T2 = EG * k2 * 128
    T4 = G1 * k4 * 128

    Wf = np.asarray(inputs["W_w"], np.float32)
    W2 = np.asarray(inputs["W2_w"], np.float32)
    W2b = np.asarray(inputs["W2_b"], np.float32)
    Wb = np.asarray(inputs["W_b"], np.float32)
    cnte_g = np.bincount(eg, minlength=EP).astype(np.float32)

    import ml_dtypes
    Xb = np.zeros((N, 2 * D), ml_dtypes.bfloat16)
    Xb[:, :D] = X
    shared = {
        "Xb": Xb,
        "W1": np.asarray(inputs["W1_w"], np.float32),
        "W2top": np.ascontiguousarray(W2[:D]),
        "W2bot": np.ascontiguousarray(W2[D:]),
        "Ww1": np.ascontiguousarray((1.0 - ALPHA) * Wf),
        "Ww2": np.ascontiguousarray(ALPHA * Wf),
        "W1b_rep": np.tile(np.asarray(inputs["W1_b"], np.float32), (128, 1)),
        "W2bWw_rep": np.tile((1.0 - ALPHA) * (W2b @ Wf), (128, 1)),
        "Wb_rep": np.tile(Wb, (128, 1)),
        "cnte": _wrap128(cnte_g),
        "iota": np.tile(np.arange(128, dtype=np.float32), (128, 1)),
        "ident": np.eye(128, dtype=np.float32),
    }

    def fill(slots_T, cell_of_tok, kcell, order, gval, dval, vval=None):
        # slots_T: total slots; cell size kcell*128; tokens sorted by `order`.
        cell = cell_of_tok[order]
        g = gval[order]
        d = dval[order]
        if len(cell):
            newc = np.empty(len(cell), bool)
            newc[0] = True
            newc[1:] = cell[1:] != cell[:-1]
            starts = np.where(newc)[0]
            idx = np.arange(len(cell))
            cell_start = np.zeros(len(cell), np.int64)
            cell_start[starts] = idx[starts]
            cell_start = np.maximum.accumulate(cell_start)
            rank = idx - cell_start
        else:
            rank = np.zeros(0, np.int64)
        slot = cell * (kcell * 128) + rank
        assert len(slot) == 0 or rank.max() < kcell * 128
        gi = np.zeros(slots_T, np.int64)
        dl = np.full(slots_T, -1.0, np.float32)
        gi[slot] = g
        dl[slot] = d
        out = [_wrap16(gi), _wrap128(dl)]
        if vval is not None:
            vv = np.zeros(slots_T, np.float32)
            vv[slot] = vval[order]
            out.append(_wrap128(vv))
        return out

    in_maps = []
    for c in range(NC):
        lo = c * S
        m = (dr >= lo) & (dr < lo + S)
        d1 = dr[m] - lo
        c1_, v1 = dc[m], dv[m]
        b1 = c1_ // BUCKET
        cell1 = b1 * G1 + d1 // 128  # bucket-major cell id
        order1 = np.lexsort((d1, b1))
        gi1, dl1, vv1 = fill(T1, cell1, k1, order1, c1_ - b1 * BUCKET, d1 % 128, v1)

        m2 = (vx >= lo) & (vx < lo + S)
        e2, v2 = eg[m2], vx[m2] - lo
        order2 = np.lexsort((e2,))
        gi2, dl2 = fill(T2, e2 // 128, k2, order2, v2, e2 % 128)
        order4 = np.lexsort((v2,))
        gi4, dl4 = fill(T4, v2 // 128, k4, order4, e2, v2 % 128)

        cntv = np.bincount(v2, minlength=SP).astype(np.float32)
        in_maps.append(dict(shared,
                            gidx1=gi1, drel1=dl1, val1=vv1,
                            gidx2=gi2, drel2=dl2,
                            gidx4=gi4, drel4=dl4,
                            cntv=_wrap128(cntv)))

    meta = dict(N=N, S=S, G1=G1, SP=SP, EG=EG, EP=EP, NB=NB,
                K1=k1, K2=k2, K4=k4, T1=T1, T2=T2, T4=T4)
    return meta, in_maps


def _build(meta):
    from concourse import bass, bacc, tile, mybir

    f32, i16 = mybir.dt.float32, mybir.dt.int16
    N, S, G1, SP, EG, EP, NB = (meta[k] for k in
                                ("N", "S", "G1", "SP", "EG", "EP", "NB"))
    K1, K2, K4, T1, T2, T4 = (meta[k] for k in
                              ("K1", "K2", "K4", "T1", "T2", "T4"))

    nc = bacc.Bacc("TRN2", target_bir_lowering=False, debug=False,
                   num_devices=NC, num_swdge_queues=4)

    def par(name, shape, dt=f32, out=False):
        return nc.declare_dram_parameter(name, list(shape), dt, isOutput=out)

    Xb = par("Xb", (N, 2 * D), mybir.dt.bfloat16)
    gidx1 = par("gidx1", (128, T1 // 16), i16)
    drel1 = par("drel1", (128, T1 // 128))
    val1 = par("val1", (128, T1 // 128))
    gidx2 = par("gidx2", (128, T2 // 16), i16)
    drel2 = par("drel2", (128, T2 // 128))
    gidx4 = par("gidx4", (128, T4 // 16), i16)
    drel4 = par("drel4", (128, T4 // 128))
    cntv = par("cntv", (128, G1))
    cnte = par("cnte", (128, EG))
    W1 = par("W1", (D, D))
    W2top = par("W2top", (D, D))
    W2bot = par("W2bot", (D, D))
    Ww1 = par("Ww1", (D, D))
    Ww2 = par("Ww2", (D, D))
    W1b_rep = par("W1b_rep", (128, D))
    W2bWw_rep = par("W2bWw_rep", (128, D))
    Wb_rep = par("Wb_rep", (128, D))
    iota = par("iota", (128, 128))
    ident = par("ident", (128, 128))
    OUT = par("OUT", (SP, D), out=True)

    eq = mybir.AluOpType.is_equal
    mult = mybir.AluOpType.mult
    addop = mybir.AluOpType.add

    with tile.TileContext(nc) as tc:
        with (
            tc.tile_pool(name="meta1", bufs=1) as metap,
            tc.tile_pool(name="gidxp", bufs=2) as gidxp,
            tc.tile_pool(name="gpool", bufs=6) as gpool,
            tc.tile_pool(name="mpool", bufs=2) as mpool,
            tc.tile_pool(name="psA", bufs=3, space="PSUM") as psA,
            tc.tile_pool(name="psT", bufs=2, space="PSUM") as psT,
            tc.tile_pool(name="small", bufs=3) as small,
            tc.tile_pool(name="stage", bufs=2) as stage,
            tc.tile_pool(name="dram", bufs=1, space="DRAM") as dram,
        ):
            # --- resident metadata ---
            def load(ap_param, shape, nm, dt=f32, pool=metap):
                t = pool.tile(list(shape), dt, name=nm, tag=nm)
                nc.scalar.dma_start(t[:], ap_param[:])
                return t

            iota_t = load(iota, (128, 128), "iota_t")
            ident_t = load(ident, (128, 128), "ident_t")
            w1_t = load(W1, (D, D), "w1_t")
            w2top_t = load(W2top, (D, D), "w2top_t")
            w2bot_t = load(W2bot, (D, D), "w2bot_t")
            ww1_t = load(Ww1, (D, D), "ww1_t")
            ww2_t = load(Ww2, (D, D), "ww2_t")
            w1b_t = load(W1b_rep, (128, D), "w1b_t")
            w2bww_t = load(W2bWw_rep, (128, D), "w2bww_t")
            wb_t = load(Wb_rep, (128, D), "wb_t")
            cntv_t = load(cntv, (128, G1), "cntv_t")
            cnte_t = load(cnte, (128, EG), "cnte_t")
            drel1_t = load(drel1, (128, T1 // 128), "drel1_t")
            val1_t = load(val1, (128, T1 // 128), "val1_t")
            drel2_t = load(drel2, (128, T2 // 128), "drel2_t")
            drel4_t = load(drel4, (128, T4 // 128), "drel4_t")

            Xd_sb = metap.tile([128, G1, D], f32)    # wrapped node shard
            B_sb = metap.tile([128, G1, D], f32)

            bf16 = mybir.dt.bfloat16
            XdB_sb = metap.tile([128, G1, D], bf16)
            Xd_hbm = dram.tile([SP, 2 * D], bf16)
            Xe_hbm = dram.tile([EP, 2 * D], bf16)
            A_part = dram.tile([EP, D], f32)
            A_full = dram.tile([EP, D], f32)
            qctr = [0]

            def sparse_step(gidx_par, gidx_cols, drel_t, val_t, srcs,
                            ngrp, kt, evac):
                """srcs: list of (src_ap, slots) bucket streams; total = sum."""
                gidx_t = gidxp.tile([128, gidx_cols], i16, tag="gidx")
                nc.scalar.dma_start(gidx_t[:], gidx_par[:])
                tile_src = {}
                base = 0
                for src_ap, L in srcs:
                    off = 0
                    while off < L:
                        n = min(CHUNK, L - off)
                        cols = n // 128
                        gt = gpool.tile([128, CHUNK // 128, 2 * D], bf16,
                                        tag="g")
                        nc.gpsimd.dma_gather(
                            gt[:, :cols, :], src_ap,
                            gidx_t[:, (base + off) // 16:(base + off + n) // 16],
                            n, n, 2 * D, queue_num=qctr[0] % 4)
                        qctr[0] += 1
                        if val_t is not None:
                            g2 = gpool.tile([128, CHUNK // 128, D], bf16,
                                            tag="g2")
                            vs = val_t[:, (base + off) // 128:
                                       (base + off) // 128 + cols]
                            nc.vector.tensor_mul(
                                g2[:, :cols, :], gt[:, :cols, :D],
                                vs.unsqueeze(2).broadcast_to([128, cols, D]))
                            src_t = (g2, D)
                        else:
                            src_t = (gt, 2 * D)
                        for i in range(cols):
                            tile_src[(base + off) // 128 + i] = (src_t[0], i,
                                                                 src_t[1])
                        off += n
                    base += L
                ntiles = base // 128
                m_buf = None
                cur = None
                for t in range(ntiles):
                    if t % MB == 0:
                        k = min(MB, ntiles - t)
                        m_buf = mpool.tile([128, MB, 128], bf16, tag="m")
                        ib = iota_t[:].unsqueeze(1).broadcast_to([128, k, 128])
                        db = drel_t[:, t:t + k].unsqueeze(2).broadcast_to(
                            [128, k, 128])
                        nc.vector.tensor_tensor(m_buf[:, :k, :], ib, db, eq)
                    i = t % kt
                    if i == 0:
                        cur = psA.tile([128, D], f32, tag="acc")
                    gt, col, w = tile_src[t]
                    nc.tensor.matmul(cur[:], m_buf[:, t % MB, :],
                                     gt[:, col, :D],
                                     start=(i == 0), stop=(i == kt - 1))
                    if i == kt - 1:
                        evac(t // kt, cur)

            # ---- step 1: diffusion into Xd ----
            bl1 = G1 * K1 * 128
            srcs1 = []
            for b in range(NB):
                rows = min(BUCKET, N - b * BUCKET)
                srcs1.append((Xb[b * BUCKET:b * BUCKET + rows, :], bl1))

            def evac1(cellidx, psum):
                b, g = divmod(cellidx, G1)
                if NB == 1:
                    nc.vector.tensor_copy(Xd_sb[:, g, :], psum[:])
                elif b == 0:
                    nc.vector.tensor_copy(Xd_sb[:, g, :], psum[:])
                else:
                    nc.vector.tensor_add(Xd_sb[:, g, :], Xd_sb[:, g, :],
                                         psum[:])
                if b == NB - 1:
                    nc.vector.tensor_copy(XdB_sb[:, g, :], Xd_sb[:, g, :])

            sparse_step(gidx1, T1 // 16, drel1_t, val1_t, srcs1, G1, K1, evac1)

            # Xd wrapped -> row-major bf16 HBM table (step-2 gather source)
            nc.sync.dma_start(
                Xd_hbm[:, :D].rearrange("(g p) f -> p g f", p=128), XdB_sb[:])

            # ---- step 2: A[e] partials ----
            ev2 = {}

            def evac2(g, psum):
                b = g % 4
                if b == 0:
                    ev2["t"] = stage.tile([128, 4, D], f32, tag="ev2", name="ev2t")
                    ev2["g0"] = g
                nc.vector.tensor_copy(ev2["t"][:, b, :], psum[:])
                if b == 3 or g == EG - 1:
                    nb = b + 1
                    nc.sync.dma_start(
                        A_part[ev2["g0"] * 128:(ev2["g0"] + nb) * 128, :]
                        .rearrange("(b p) f -> p b f", p=128),
                        ev2["t"][:, :nb, :])

            sparse_step(gidx2, T2 // 16, drel2_t, None,
                        [(Xd_hbm[:, :], T2)], EG, K2, evac2)

            # ---- AllReduce A ----
            nc.gpsimd.collective_compute(
                "AllReduce", addop,
                replica_groups=[list(range(NC))],
                ins=[A_part.opt()], outs=[A_full.opt()])

            # ---- step 3: Xe = A @ W1 + cnt_e x b1 ----
            ev3 = {}
            for g in range(EG):
                a_t = stage.tile([128, D], f32, tag="a")
                nc.scalar.dma_start(a_t[:], A_full[g * 128:(g + 1) * 128, :])
                pT = psT.tile([D, 128], f32, tag="t")
                nc.tensor.transpose(pT[:], a_t[:], ident_t[:])
                aT = stage.tile([D, 128], f32, tag="aT")
                nc.vector.tensor_copy(aT[:], pT[:])
                p2 = psA.tile([128, D], f32, tag="acc")
                nc.tensor.matmul(p2[:], aT[:], w1_t[:], start=True, stop=True)
                b4 = g % 4
                if b4 == 0:
                    ev3["t"] = stage.tile([128, 4, D], bf16, tag="ev3",
                                          name="ev3t")
                    ev3["g0"] = g
                nc.vector.scalar_tensor_tensor(
                    ev3["t"][:, b4, :], w1b_t[:], cnte_t[:, g:g + 1], p2[:],
                    mult, addop)
                if b4 == 3 or g == EG - 1:
                    nb = b4 + 1
                    nc.sync.dma_start(
                        Xe_hbm[ev3["g0"] * 128:(ev3["g0"] + nb) * 128, :D]
                        .rearrange("(b p) f -> p b f", p=128),
                        ev3["t"][:, :nb, :])

            # ---- step 4: B[v] ----
            def evac4(g, psum):
                nc.vector.tensor_copy(B_sb[:, g, :], psum[:])

            sparse_step(gidx4, T4 // 16, drel4_t, None,
                        [(Xe_hbm[:, :], T4)], G1, K4, evac4)

            # ---- steps 5-7 ----
            for g in range(G1):
                xd = Xd_sb[:, g, :]
                cnt = cntv_t[:, g:g + 1]
                xdc = stage.tile([128, D], f32, tag="xdc")
                nc.vector.tensor_scalar_mul(xdc[:], xd, cnt)
                pT1 = psT.tile([D, 128], f32, tag="t")
                nc.tensor.transpose(pT1[:], xdc[:], ident_t[:])
                xdcT = stage.tile([D, 128], f32, tag="xdcT")
                nc.vector.tensor_copy(xdcT[:], pT1[:])
                pT2 = psT.tile([D, 128], f32, tag="t")
                nc.tensor.transpose(pT2[:], xd, ident_t[:])
                xdT = stage.tile([D, 128], f32, tag="xdT")
                nc.vector.tensor_copy(xdT[:], pT2[:])
                pT3 = psT.tile([D, 128], f32, tag="t")
                nc.tensor.transpose(pT3[:], B_sb[:, g, :], ident_t[:])
                bT = stage.tile([D, 128], f32, tag="bT")
                nc.vector.tensor_copy(bT[:], pT3[:])
                pvT = psT.tile([D, 128], f32, tag="vt")
                nc.tensor.matmul(pvT[:], w2top_t[:], xdcT[:],
                                 start=True, stop=False)
                nc.tensor.matmul(pvT[:], w2bot_t[:], bT[:],
                                 start=False, stop=True)
                xvT = stage.tile([D, 128], f32, tag="xvT")
                nc.vector.tensor_copy(xvT[:], pvT[:])
                po = psA.tile([128, D], f32, tag="acc")
                nc.tensor.matmul(po[:], xvT[:], ww1_t[:],
                                 start=True, stop=False)
                nc.tensor.matmul(po[:], xdT[:], ww2_t[:],
                                 start=False, stop=True)
                tmp = stage.tile([128, D], f32, tag="tmp")
                nc.vector.scalar_tensor_tensor(
                    tmp[:], w2bww_t[:], cnt, po[:], mult, addop)
                ot = stage.tile([128, D], f32, tag="ot")
                nc.vector.tensor_add(ot[:], tmp[:], wb_t[:])
                nc.sync.dma_start(OUT[g * 128:(g + 1) * 128, :], ot[:])

    nc.compile()
    return nc


def _run(inputs, n_edges, sim=False):
    meta, in_maps = _prep(inputs, n_edges)
    nc = _build(meta)
    S, SP = meta["S"], meta["SP"]
    if sim:
        from concourse import bass_interp
        ms = bass_interp.MultiCoreSim(nc, NC, require_finite=False, require_nnan=False)
        for c in range(NC):
            for k, v in in_maps[c].items():
                ms.cores[c].tensor(k)[:] = v
        ms.simulate()
        outs = [np.array(ms.cores[c].mem_tensor("OUT")).reshape(SP, D)
                for c in range(NC)]
    else:
        from concourse.bass_utils import run_bass_kernel_spmd
        res = run_bass_kernel_spmd(nc, in_maps, list(range(NC)),
                                   trace=TRACE)
        global LAST_EXEC_NS, LAST_RESULTS
        LAST_EXEC_NS = res.exec_time_ns
        LAST_RESULTS = res
        outs = [res.results[c]["OUT"] for c in range(NC)]
    return np.concatenate([o[:S] for o in outs], axis=0).astype(np.float32)


def kernel(**inputs):
    return _run(inputs, 25000, sim=False)


# revision 15
# speedup vs baseline: 3.4100x; 1.7710x over previous
"""EquivSetConv (hypergraph message passing) Trainium2 Bass kernel.

Math (reference):
  Xd = segment_sum(dif_vals * X[dif_cols], dif_rows, N)
  Xe = segment_sum((Xd@W1+b1)[vertex], edges, E)
  Xv = segment_sum(concat(Xd[vertex], Xe[edges]) @ W2 + b2, vertex, N)
  out = ((1-a)*Xv + a*Xd) @ W + b

Algebraic reassociation used here (exact up to fp reassociation):
  A[e]  = segment_sum(Xd[vertex], edges, E)
  Xe    = A @ W1 + cnt_e x b1
  B[v]  = segment_sum(Xe[edges], vertex, N)
  Xv    = cnt_v * (Xd @ W2top) + B @ W2bot + cnt_v x b2
  out   = ((1-a)Xv + a Xd) @ W + b

Distribution: nodes sharded 8 ways by row range; incidence lists bucketed by
destination core; the only collective is an AllReduce of the per-core partial
A [E,64]. Segment sums run as one-hot matmul accumulation in PSUM over
128-destination groups; gathers use the MoE dma_gather (int16 idx, <=1024/call).
"""
import sys
import numpy as np

sys.path.insert(0, "/opt/trn_rl_repo")

D = 64
NC = 8
CHUNK = 1024        # dma_gather per-call token cap
MB = 16             # one-hot tiles built per DVE op
ALPHA = 0.5
BUCKET = 32768      # int16 gather index range
TRACE = False
LAST_EXEC_NS = None
LAST_RESULTS = None


def _wrap16(a):
    a = np.asarray(a, np.int16)
    return np.tile(a.reshape(-1, 16).T, (8, 1))  # [128, T/16]


def _wrap128(a):
    return np.ascontiguousarray(np.asarray(a).reshape(-1, 128).T)  # [128, T/128]


def _prep(inputs, n_edges):
    X = np.ascontiguousarray(np.asarray(inputs["X"], np.float32))
    N = X.shape[0]
    assert N % NC == 0
    S = N // NC
    G1 = -(-S // 128)          # node groups per core
    SP = G1 * 128
    EG = -(-n_edges // 128)    # edge groups
    EP = EG * 128
    NB = -(-N // BUCKET)

    dr = np.asarray(inputs["dif_rows"], np.int64)
    dc = np.asarray(inputs["dif_cols"], np.int64)
    dv = np.asarray(inputs["dif_vals"], np.float32)
    vx = np.asarray(inputs["vertex"], np.int64)
    eg = np.asarray(inputs["edges"], np.int64)
    assert eg.max() < n_edges and vx.max() < N and dr.max() < N and dc.max() < N

    # --- per-cell tile plans (max fill over cores; uniform across cores) ---
    def plan(core, cell, ncells, min_one):
        cnt = np.bincount(core * ncells + cell,
                          minlength=NC * ncells).reshape(NC, ncells)
        k = -(-cnt.max(0) // 128)
        k = np.maximum(k, min_one)
        off = np.zeros(ncells + 1, np.int64)
        np.cumsum(k, out=off[1:])
        return k, off * 128, int(off[-1]) * 128

    c1 = dr // S
    min1 = np.zeros(NB * G1, np.int64)
    min1[:G1] = 1  # bucket-0 cells init the Xd accumulator
    kc1, off1, T1 = plan(c1, (dc // BUCKET) * G1 + (dr % S) // 128,
                         NB * G1, min1)
    c2 = vx // S
    kc2, off2, T2 = plan(c2, eg // 128, EG, 1)
    kc4, off4, T4 = plan(c2, (vx % S) // 128, G1, 1)
    T1 = -(-T1 // 2048) * 2048  # keep /16 and /128 wrappings integral
    T2 = -(-T2 // 2048) * 2048
    T4 = -(-T4 // 2048) * 2048

    Wf = np.asarray(inputs["W_w"], np.float32)
    W2 = np.asarray(inputs["W2_w"], np.float32)
    W2b = np.asarray(inputs["W2_b"], np.float32)
    Wb = np.asarray(inputs["W_b"], np.float32)
    cnte_g = np.bincount(eg, minlength=EP).astype(np.float32)

    import ml_dtypes
    Xb = np.zeros((N, 2 * D), ml_dtypes.bfloat16)
    Xb[:, :D] = X
    shared = {
        "Xb": Xb,
        "W1": np.asarray(inputs["W1_w"], np.float32),
        "W2top": np.ascontiguousarray(W2[:D]),
        "W2bot": np.ascontiguousarray(W2[D:]),
        "Ww1": np.ascontiguousarray((1.0 - ALPHA) * Wf),
        "Ww2": np.ascontiguousarray(ALPHA * Wf),
        "W1b_rep": np.tile(np.asarray(inputs["W1_b"], np.float32), (128, 1)),
        "W2bWw_rep": np.tile((1.0 - ALPHA) * (W2b @ Wf), (128, 1)),
        "Wb_rep": np.tile(Wb, (128, 1)),
        "cnte": _wrap128(cnte_g),
        "iota": np.tile(np.arange(128, dtype=ml_dtypes.bfloat16), (128, 1)),
        "ident": np.eye(128, dtype=np.float32),
    }

    def fill(slots_T, cell_of_tok, kcell, offs, order, gval, dval, vval=None):
        # slots_T: total slots; cell size kcell*128; tokens sorted by `order`.
        cell = cell_of_tok[order]
        g = gval[order]
        d = dval[order]
        if len(cell):
            newc = np.empty(len(cell), bool)
            newc[0] = True
            newc[1:] = cell[1:] != cell[:-1]
            starts = np.where(newc)[0]
            idx = np.arange(len(cell))
            cell_start = np.zeros(len(cell), np.int64)
            cell_start[starts] = idx[starts]
            cell_start = np.maximum.accumulate(cell_start)
            rank = idx - cell_start
        else:
            rank = np.zeros(0, np.int64)
        slot = offs[cell] + rank
        assert len(slot) == 0 or (rank < kcell[cell] * 128).all()
        gi = np.zeros(slots_T, np.int64)
        dl = np.full(slots_T, -1.0, np.float32)
        gi[slot] = g
        dl[slot] = d
        import ml_dtypes as _md
        out = [_wrap16(gi), _wrap128(dl.astype(_md.bfloat16))]
        if vval is not None:
            vv = np.zeros(slots_T, np.float32)
            vv[slot] = vval[order]
            out.append(_wrap128(vv))
        return out

    in_maps = []
    for c in range(NC):
        lo = c * S
        m = (dr >= lo) & (dr < lo + S)
        d1 = dr[m] - lo
        c1_, v1 = dc[m], dv[m]
        b1 = c1_ // BUCKET
        cell1 = b1 * G1 + d1 // 128  # bucket-major cell id
        order1 = np.lexsort((d1, b1))
        gi1, dl1, vv1 = fill(T1, cell1, kc1, off1, order1, c1_ - b1 * BUCKET, d1 % 128, v1)

        m2 = (vx >= lo) & (vx < lo + S)
        e2, v2 = eg[m2], vx[m2] - lo
        order2 = np.lexsort((e2,))
        gi2, dl2 = fill(T2, e2 // 128, kc2, off2, order2, v2, e2 % 128)
        order4 = np.lexsort((v2,))
        gi4, dl4 = fill(T4, v2 // 128, kc4, off4, order4, e2, v2 % 128)

        cntv = np.bincount(v2, minlength=SP).astype(np.float32)
        in_maps.append(dict(shared,
                            gidx1=gi1, drel1=dl1, val1=vv1,
                            gidx2=gi2, drel2=dl2,
                            gidx4=gi4, drel4=dl4,
                            cntv=_wrap128(cntv)))

    meta = dict(N=N, S=S, G1=G1, SP=SP, EG=EG, EP=EP, NB=NB,
                KC1=kc1.tolist(), OFF1=off1.tolist(),
                KC2=kc2.tolist(), OFF2=off2.tolist(),
                KC4=kc4.tolist(), OFF4=off4.tolist(),
                T1=T1, T2=T2, T4=T4)
    return meta, in_maps


def _build(meta):
    from concourse import bass, bacc, tile, mybir

    f32, i16 = mybir.dt.float32, mybir.dt.int16
    N, S, G1, SP, EG, EP, NB = (meta[k] for k in
                                ("N", "S", "G1", "SP", "EG", "EP", "NB"))
    T1, T2, T4 = meta["T1"], meta["T2"], meta["T4"]
    KC1, OFF1 = meta["KC1"], meta["OFF1"]
    KC2, OFF2 = meta["KC2"], meta["OFF2"]
    KC4, OFF4 = meta["KC4"], meta["OFF4"]

    nc = bacc.Bacc("TRN2", target_bir_lowering=False, debug=False,
                   num_devices=NC, num_swdge_queues=4)

    def par(name, shape, dt=f32, out=False):
        return nc.declare_dram_parameter(name, list(shape), dt, isOutput=out)

    Xb = par("Xb", (N, 2 * D), mybir.dt.bfloat16)
    bf16p = mybir.dt.bfloat16
    gidx1 = par("gidx1", (128, T1 // 16), i16)
    drel1 = par("drel1", (128, T1 // 128), bf16p)
    val1 = par("val1", (128, T1 // 128))
    gidx2 = par("gidx2", (128, T2 // 16), i16)
    drel2 = par("drel2", (128, T2 // 128), bf16p)
    gidx4 = par("gidx4", (128, T4 // 16), i16)
    drel4 = par("drel4", (128, T4 // 128), bf16p)
    cntv = par("cntv", (128, G1))
    cnte = par("cnte", (128, EG))
    W1 = par("W1", (D, D))
    W2top = par("W2top", (D, D))
    W2bot = par("W2bot", (D, D))
    Ww1 = par("Ww1", (D, D))
    Ww2 = par("Ww2", (D, D))
    W1b_rep = par("W1b_rep", (128, D))
    W2bWw_rep = par("W2bWw_rep", (128, D))
    Wb_rep = par("Wb_rep", (128, D))
    iota = par("iota", (128, 128), mybir.dt.bfloat16)
    ident = par("ident", (128, 128))
    OUT = par("OUT", (SP, D), out=True)

    eq = mybir.AluOpType.is_equal
    mult = mybir.AluOpType.mult
    addop = mybir.AluOpType.add

    with tile.TileContext(nc) as tc:
        with (
            tc.tile_pool(name="meta1", bufs=1) as metap,
            tc.tile_pool(name="gidxp", bufs=2) as gidxp,
            tc.tile_pool(name="gpool", bufs=10) as gpool,
            tc.tile_pool(name="mpool", bufs=2) as mpool,
            tc.tile_pool(name="psA", bufs=3, space="PSUM") as psA,
            tc.tile_pool(name="psT", bufs=2, space="PSUM") as psT,
            tc.tile_pool(name="small", bufs=3) as small,
            tc.tile_pool(name="stage", bufs=2) as stage,
            tc.tile_pool(name="dram", bufs=1, space="DRAM") as dram,
        ):
            # --- resident metadata ---
            def load(ap_param, shape, nm, dt=f32, pool=metap):
                t = pool.tile(list(shape), dt, name=nm, tag=nm)
                nc.scalar.dma_start(t[:], ap_param[:])
                return t

            iota_t = load(iota, (128, 128), "iota_t",
                          dt=mybir.dt.bfloat16)
            ident_t = load(ident, (128, 128), "ident_t")
            w1_t = load(W1, (D, D), "w1_t")
            w2top_t = load(W2top, (D, D), "w2top_t")
            w2bot_t = load(W2bot, (D, D), "w2bot_t")
            ww1_t = load(Ww1, (D, D), "ww1_t")
            ww2_t = load(Ww2, (D, D), "ww2_t")
            w1b_t = load(W1b_rep, (128, D), "w1b_t")
            w2bww_t = load(W2bWw_rep, (128, D), "w2bww_t")
            wb_t = load(Wb_rep, (128, D), "wb_t")
            cntv_t = load(cntv, (128, G1), "cntv_t")
            cnte_t = load(cnte, (128, EG), "cnte_t")
            drel1_t = load(drel1, (128, T1 // 128), "drel1_t",
                           dt=mybir.dt.bfloat16)
            val1_t = load(val1, (128, T1 // 128), "val1_t")
            drel2_t = load(drel2, (128, T2 // 128), "drel2_t",
                           dt=mybir.dt.bfloat16)
            drel4_t = load(drel4, (128, T4 // 128), "drel4_t",
                           dt=mybir.dt.bfloat16)

            Xd_sb = metap.tile([128, G1, D], f32)    # wrapped node shard
            B_sb = metap.tile([128, G1, D], f32)

            bf16 = mybir.dt.bfloat16
            XdB_sb = metap.tile([128, G1, D], bf16)
            Xd_hbm = dram.tile([SP, 2 * D], bf16)
            Xe_hbm = dram.tile([EP, 2 * D], bf16)
            A_part = dram.tile([EP, D], f32)
            A_full = dram.tile([EP, D], f32, addr_space="Shared")
            qctr = [0]

            def sparse_step(gidx_par, gidx_cols, drel_t, val_t, srcs,
                            kcells, offs, evac):
                """srcs: (src_ap, cell_lo, cell_hi) bucket streams covering
                cells [lo, hi); slot spans from offs."""
                gidx_t = gidxp.tile([128, gidx_cols], i16, tag="gidx")
                nc.scalar.dma_start(gidx_t[:], gidx_par[:])
                tile_src = {}
                for src_ap, c_lo, c_hi in srcs:
                    base, end = offs[c_lo], offs[c_hi]
                    off = 0
                    L = end - base
                    while off < L:
                        n = min(CHUNK, L - off)
                        cols = n // 128
                        gt = gpool.tile([128, CHUNK // 128, 2 * D], bf16,
                                        tag="g")
                        nc.gpsimd.dma_gather(
                            gt[:, :cols, :], src_ap,
                            gidx_t[:, (base + off) // 16:(base + off + n) // 16],
                            n, n, 2 * D, queue_num=qctr[0] % 4)
                        qctr[0] += 1
                        if val_t is not None:
                            g2 = gpool.tile([128, CHUNK // 128, D], bf16,
                                            tag="g2")
                            vs = val_t[:, (base + off) // 128:
                                       (base + off) // 128 + cols]
                            nc.vector.tensor_mul(
                                g2[:, :cols, :], gt[:, :cols, :D],
                                vs.unsqueeze(2).broadcast_to([128, cols, D]))
                            src_t = (g2, D)
                        else:
                            src_t = (gt, 2 * D)
                        for i in range(cols):
                            tile_src[(base + off) // 128 + i] = (src_t[0], i)
                        off += n
                ntiles = offs[len(kcells)] // 128
                mb_next = 0
                m_buf = None
                for cell in range(len(kcells)):
                    kt = kcells[cell]
                    if kt == 0:
                        continue
                    cur = psA.tile([128, D], f32, tag="acc")
                    t0 = offs[cell] // 128
                    for i in range(kt):
                        t = t0 + i
                        if t >= mb_next:
                            k = min(MB, ntiles - t)
                            m_buf = mpool.tile([128, MB, 128], bf16, tag="m")
                            ib = iota_t[:].unsqueeze(1).broadcast_to(
                                [128, k, 128])
                            db = drel_t[:, t:t + k].unsqueeze(2).broadcast_to(
                                [128, k, 128])
                            nc.vector.tensor_tensor(m_buf[:, :k, :], ib, db,
                                                    eq)
                            mb_base, mb_next = t, t + k
                        gt, col = tile_src[t]
                        nc.tensor.matmul(cur[:], m_buf[:, t - mb_base, :],
                                         gt[:, col, :D],
                                         start=(i == 0), stop=(i == kt - 1))
                    evac(cell, cur)

            # ---- step 1: diffusion into Xd ----
            srcs1 = []
            for b in range(NB):
                rows = min(BUCKET, N - b * BUCKET)
                srcs1.append((Xb[b * BUCKET:b * BUCKET + rows, :],
                              b * G1, (b + 1) * G1))

            def evac1(cellidx, psum):
                b, g = divmod(cellidx, G1)
                if b == 0:
                    nc.vector.tensor_copy(Xd_sb[:, g, :], psum[:])
                else:
                    nc.vector.tensor_add(Xd_sb[:, g, :], Xd_sb[:, g, :],
                                         psum[:])

            sparse_step(gidx1, T1 // 16, drel1_t, val1_t, srcs1, KC1, OFF1, evac1)

            for g in range(G1):
                nc.vector.tensor_copy(XdB_sb[:, g, :], Xd_sb[:, g, :])
            # Xd wrapped -> row-major bf16 HBM table (step-2 gather source)
            nc.sync.dma_start(
                Xd_hbm[:, :D].rearrange("(g p) f -> p g f", p=128), XdB_sb[:])

            # ---- step 2: A[e] partials ----
            ev2 = {}

            def evac2(g, psum):
                b = g % 4
                if b == 0:
                    ev2["t"] = stage.tile([128, 4, D], f32, tag="ev2", name="ev2t")
                    ev2["g0"] = g
                nc.vector.tensor_copy(ev2["t"][:, b, :], psum[:])
                if b == 3 or g == EG - 1:
                    nb = b + 1
                    nc.sync.dma_start(
                        A_part[ev2["g0"] * 128:(ev2["g0"] + nb) * 128, :]
                        .rearrange("(b p) f -> p b f", p=128),
                        ev2["t"][:, :nb, :])

            sparse_step(gidx2, T2 // 16, drel2_t, None,
                        [(Xd_hbm[:, :], 0, EG)], KC2, OFF2, evac2)

            # ---- AllReduce A ----
            nc.gpsimd.collective_compute(
                "AllReduce", addop,
                replica_groups=[list(range(NC))],
                ins=[A_part.opt()], outs=[A_full.opt()])

            # ---- step 3: Xe = A @ W1 + cnt_e x b1 ----
            ev3 = {}
            for g in range(EG):
                a_t = stage.tile([128, D], f32, tag="a")
                nc.scalar.dma_start(a_t[:], A_full[g * 128:(g + 1) * 128, :])
                pT = psT.tile([D, 128], f32, tag="t")
                nc.tensor.transpose(pT[:], a_t[:], ident_t[:])
                aT = stage.tile([D, 128], f32, tag="aT")
                nc.vector.tensor_copy(aT[:], pT[:])
                p2 = psA.tile([128, D], f32, tag="acc")
                nc.tensor.matmul(p2[:], aT[:], w1_t[:], start=True, stop=True)
                b4 = g % 4
                if b4 == 0:
                    ev3["t"] = stage.tile([128, 4, D], bf16, tag="ev3",
                                          name="ev3t")
                    ev3["g0"] = g
                nc.vector.scalar_tensor_tensor(
                    ev3["t"][:, b4, :], w1b_t[:], cnte_t[:, g:g + 1], p2[:],
                    mult, addop)
                if b4 == 3 or g == EG - 1:
                    nb = b4 + 1
                    nc.sync.dma_start(
                        Xe_hbm[ev3["g0"] * 128:(ev3["g0"] + nb) * 128, :D]
                        .rearrange("(b p) f -> p b f", p=128),
                        ev3["t"][:, :nb, :])

            # ---- step 4: B[v] ----
            def evac4(g, psum):
                nc.vector.tensor_copy(B_sb[:, g, :], psum[:])

            sparse_step(gidx4, T4 // 16, drel4_t, None,
                        [(Xe_hbm[:, :], 0, G1)], KC4, OFF4, evac4)

            # ---- steps 5-7 ----
            for g in range(G1):
                xd = Xd_sb[:, g, :]
                cnt = cntv_t[:, g:g + 1]
                xdc = stage.tile([128, D], f32, tag="xdc")
                nc.vector.tensor_scalar_mul(xdc[:], xd, cnt)
                pT1 = psT.tile([D, 128], f32, tag="t")
                nc.tensor.transpose(pT1[:], xdc[:], ident_t[:])
                xdcT = stage.tile([D, 128], f32, tag="xdcT")
                nc.vector.tensor_copy(xdcT[:], pT1[:])
                pT2 = psT.tile([D, 128], f32, tag="t")
                nc.tensor.transpose(pT2[:], xd, ident_t[:])
                xdT = stage.tile([D, 128], f32, tag="xdT")
                nc.vector.tensor_copy(xdT[:], pT2[:])
                pT3 = psT.tile([D, 128], f32, tag="t")
                nc.tensor.transpose(pT3[:], B_sb[:, g, :], ident_t[:])
                bT = stage.tile([D, 128], f32, tag="bT")
                nc.vector.tensor_copy(bT[:], pT3[:])
                pvT = psT.tile([D, 128], f32, tag="vt")
                nc.tensor.matmul(pvT[:], w2top_t[:], xdcT[:],
                                 start=True, stop=False)
                nc.tensor.matmul(pvT[:], w2bot_t[:], bT[:],
                                 start=False, stop=True)
                xvT = stage.tile([D, 128], f32, tag="xvT")
                nc.vector.tensor_copy(xvT[:], pvT[:])
                po = psA.tile([128, D], f32, tag="acc")
                nc.tensor.matmul(po[:], xvT[:], ww1_t[:],
                                 start=True, stop=False)
                nc.tensor.matmul(po[:], xdT[:], ww2_t[:],
                                 start=False, stop=True)
                tmp = stage.tile([128, D], f32, tag="tmp")
                nc.vector.scalar_tensor_tensor(
                    tmp[:], w2bww_t[:], cnt, po[:], mult, addop)
                ot = stage.tile([128, D], f32, tag="ot")
                nc.vector.tensor_add(ot[:], tmp[:], wb_t[:])
                nc.sync.dma_start(OUT[g * 128:(g + 1) * 128, :], ot[:])

    nc.compile()
    return nc


def _run(inputs, n_edges, sim=False):
    meta, in_maps = _prep(inputs, n_edges)
    nc = _build(meta)
    S, SP = meta["S"], meta["SP"]
    if sim:
        from concourse import bass_interp
        ms = bass_interp.MultiCoreSim(nc, NC, require_finite=False, require_nnan=False)
        for c in range(NC):
            for k, v in in_maps[c].items():
                ms.cores[c].tensor(k)[:] = v
        ms.simulate()
        outs = [np.array(ms.cores[c].mem_tensor("OUT")).reshape(SP, D)
                for c in range(NC)]
    else:
        from concourse.bass_utils import run_bass_kernel_spmd
        res = run_bass_kernel_spmd(nc, in_maps, list(range(NC)),
                                   trace=TRACE)
        global LAST_EXEC_NS, LAST_RESULTS
        LAST_EXEC_NS = res.exec_time_ns
        LAST_RESULTS = res
        outs = [res.results[c]["OUT"] for c in range(NC)]
    return np.concatenate([o[:S] for o in outs], axis=0).astype(np.float32)


def kernel(**inputs):
    return _run(inputs, 25000, sim=False)


# revision 17
# speedup vs baseline: 3.7263x; 1.0928x over previous
"""EquivSetConv (hypergraph message passing) Trainium2 Bass kernel.

Math (reference):
  Xd = segment_sum(dif_vals * X[dif_cols], dif_rows, N)
  Xe = segment_sum((Xd@W1+b1)[vertex], edges, E)
  Xv = segment_sum(concat(Xd[vertex], Xe[edges]) @ W2 + b2, vertex, N)
  out = ((1-a)*Xv + a*Xd) @ W + b

Algebraic reassociation used here (exact up to fp reassociation):
  A[e]  = segment_sum(Xd[vertex], edges, E)
  Xe    = A @ W1 + cnt_e x b1
  B[v]  = segment_sum(Xe[edges], vertex, N)
  Xv    = cnt_v * (Xd @ W2top) + B @ W2bot + cnt_v x b2
  out   = ((1-a)Xv + a Xd) @ W + b

Distribution: nodes sharded 8 ways by row range; incidence lists bucketed by
destination core; the only collective is an AllReduce of the per-core partial
A [E,64]. Segment sums run as one-hot matmul accumulation in PSUM over
128-destination groups; gathers use the MoE dma_gather (int16 idx, <=1024/call).
"""
import sys
import numpy as np

sys.path.insert(0, "/opt/trn_rl_repo")

D = 64
NC = 8
CHUNK = 1024        # dma_gather per-call token cap
MB = 16             # one-hot tiles built per DVE op
ALPHA = 0.5
BUCKET = 32768      # int16 gather index range
TRACE = False
LAST_EXEC_NS = None
LAST_RESULTS = None


def _wrap16(a):
    a = np.asarray(a, np.int16)
    return np.tile(a.reshape(-1, 16).T, (8, 1))  # [128, T/16]


def _wrap128(a):
    return np.ascontiguousarray(np.asarray(a).reshape(-1, 128).T)  # [128, T/128]


def _prep(inputs, n_edges):
    X = np.ascontiguousarray(np.asarray(inputs["X"], np.float32))
    N = X.shape[0]
    assert N % NC == 0
    S = N // NC
    G1 = -(-S // 128)          # node groups per core
    SP = G1 * 128
    EG = -(-n_edges // 128)    # edge groups
    EP = EG * 128
    NB = -(-N // BUCKET)

    dr = np.asarray(inputs["dif_rows"], np.int64)
    dc = np.asarray(inputs["dif_cols"], np.int64)
    dv = np.asarray(inputs["dif_vals"], np.float32)
    vx = np.asarray(inputs["vertex"], np.int64)
    eg = np.asarray(inputs["edges"], np.int64)
    assert eg.max() < n_edges and vx.max() < N and dr.max() < N and dc.max() < N

    # --- per-cell tile plans (max fill over cores; uniform across cores) ---
    def plan(core, cell, ncells, min_one):
        cnt = np.bincount(core * ncells + cell,
                          minlength=NC * ncells).reshape(NC, ncells)
        k = -(-cnt.max(0) // 128)
        k = np.maximum(k, min_one)
        off = np.zeros(ncells + 1, np.int64)
        np.cumsum(k, out=off[1:])
        return k, off * 128, int(off[-1]) * 128

    c1 = dr // S
    min1 = np.zeros(NB * G1, np.int64)
    min1[:G1] = 1  # bucket-0 cells init the Xd accumulator
    kc1, off1, T1 = plan(c1, (dc // BUCKET) * G1 + (dr % S) // 128,
                         NB * G1, min1)
    c2 = vx // S
    kc2, off2, T2 = plan(c2, eg // 128, EG, 1)
    kc4, off4, T4 = plan(c2, (vx % S) // 128, G1, 1)
    T1 = -(-T1 // 2048) * 2048  # keep /16 and /128 wrappings integral
    T2 = -(-T2 // 2048) * 2048
    T4 = -(-T4 // 2048) * 2048

    Wf = np.asarray(inputs["W_w"], np.float32)
    W2 = np.asarray(inputs["W2_w"], np.float32)
    W2b = np.asarray(inputs["W2_b"], np.float32)
    Wb = np.asarray(inputs["W_b"], np.float32)
    cnte_g = np.bincount(eg, minlength=EP).astype(np.float32)

    import ml_dtypes
    Xb = np.zeros((N, 2 * D), ml_dtypes.bfloat16)
    Xb[:, :D] = X
    shared = {
        "Xb": Xb,
        "W1": np.asarray(inputs["W1_w"], np.float32).astype(ml_dtypes.bfloat16),
        "W2top": np.ascontiguousarray(W2[:D]).astype(ml_dtypes.bfloat16),
        "W2bot": np.ascontiguousarray(W2[D:]).astype(ml_dtypes.bfloat16),
        "Ww1": np.ascontiguousarray((1.0 - ALPHA) * Wf).astype(ml_dtypes.bfloat16),
        "Ww2": np.ascontiguousarray(ALPHA * Wf).astype(ml_dtypes.bfloat16),
        "W1b_rep": np.tile(np.asarray(inputs["W1_b"], np.float32), (128, 1)),
        "W2bWw_rep": np.tile((1.0 - ALPHA) * (W2b @ Wf), (128, 1)),
        "Wb_rep": np.tile(Wb, (128, 1)),
        "cnte": _wrap128(cnte_g),
        "iota": np.tile(np.arange(128, dtype=ml_dtypes.bfloat16), (128, 1)),
        "ident": np.eye(128).astype(ml_dtypes.bfloat16),
    }

    def fill(slots_T, cell_of_tok, kcell, offs, order, gval, dval, vval=None):
        # slots_T: total slots; cell size kcell*128; tokens sorted by `order`.
        cell = cell_of_tok[order]
        g = gval[order]
        d = dval[order]
        if len(cell):
            newc = np.empty(len(cell), bool)
            newc[0] = True
            newc[1:] = cell[1:] != cell[:-1]
            starts = np.where(newc)[0]
            idx = np.arange(len(cell))
            cell_start = np.zeros(len(cell), np.int64)
            cell_start[starts] = idx[starts]
            cell_start = np.maximum.accumulate(cell_start)
            rank = idx - cell_start
        else:
            rank = np.zeros(0, np.int64)
        slot = offs[cell] + rank
        assert len(slot) == 0 or (rank < kcell[cell] * 128).all()
        gi = np.zeros(slots_T, np.int64)
        dl = np.full(slots_T, -1.0, np.float32)
        gi[slot] = g
        dl[slot] = d
        import ml_dtypes as _md
        out = [_wrap16(gi), _wrap128(dl.astype(_md.bfloat16))]
        if vval is not None:
            vv = np.zeros(slots_T, np.float32)
            vv[slot] = vval[order]
            out.append(_wrap128(vv))
        return out

    in_maps = []
    for c in range(NC):
        lo = c * S
        m = (dr >= lo) & (dr < lo + S)
        d1 = dr[m] - lo
        c1_, v1 = dc[m], dv[m]
        b1 = c1_ // BUCKET
        cell1 = b1 * G1 + d1 // 128  # bucket-major cell id
        order1 = np.lexsort((d1, b1))
        gi1, dl1, vv1 = fill(T1, cell1, kc1, off1, order1, c1_ - b1 * BUCKET, d1 % 128, v1)

        m2 = (vx >= lo) & (vx < lo + S)
        e2, v2 = eg[m2], vx[m2] - lo
        order2 = np.lexsort((e2,))
        gi2, dl2 = fill(T2, e2 // 128, kc2, off2, order2, v2, e2 % 128)
        order4 = np.lexsort((v2,))
        gi4, dl4 = fill(T4, v2 // 128, kc4, off4, order4, e2, v2 % 128)

        cntv = np.bincount(v2, minlength=SP).astype(np.float32)
        in_maps.append(dict(shared,
                            gidx1=gi1, drel1=dl1, val1=vv1,
                            gidx2=gi2, drel2=dl2,
                            gidx4=gi4, drel4=dl4,
                            cntv=_wrap128(cntv)))

    meta = dict(N=N, S=S, G1=G1, SP=SP, EG=EG, EP=EP, NB=NB,
                KC1=kc1.tolist(), OFF1=off1.tolist(),
                KC2=kc2.tolist(), OFF2=off2.tolist(),
                KC4=kc4.tolist(), OFF4=off4.tolist(),
                T1=T1, T2=T2, T4=T4)
    return meta, in_maps


def _build(meta):
    from concourse import bass, bacc, tile, mybir

    f32, i16 = mybir.dt.float32, mybir.dt.int16
    N, S, G1, SP, EG, EP, NB = (meta[k] for k in
                                ("N", "S", "G1", "SP", "EG", "EP", "NB"))
    T1, T2, T4 = meta["T1"], meta["T2"], meta["T4"]
    KC1, OFF1 = meta["KC1"], meta["OFF1"]
    KC2, OFF2 = meta["KC2"], meta["OFF2"]
    KC4, OFF4 = meta["KC4"], meta["OFF4"]

    nc = bacc.Bacc("TRN2", target_bir_lowering=False, debug=False,
                   num_devices=NC, num_swdge_queues=4)

    def par(name, shape, dt=f32, out=False):
        return nc.declare_dram_parameter(name, list(shape), dt, isOutput=out)

    Xb = par("Xb", (N, 2 * D), mybir.dt.bfloat16)
    bf16p = mybir.dt.bfloat16
    gidx1 = par("gidx1", (128, T1 // 16), i16)
    drel1 = par("drel1", (128, T1 // 128), bf16p)
    val1 = par("val1", (128, T1 // 128))
    gidx2 = par("gidx2", (128, T2 // 16), i16)
    drel2 = par("drel2", (128, T2 // 128), bf16p)
    gidx4 = par("gidx4", (128, T4 // 16), i16)
    drel4 = par("drel4", (128, T4 // 128), bf16p)
    cntv = par("cntv", (128, G1))
    cnte = par("cnte", (128, EG))
    W1 = par("W1", (D, D), bf16p)
    W2top = par("W2top", (D, D), bf16p)
    W2bot = par("W2bot", (D, D), bf16p)
    Ww1 = par("Ww1", (D, D), bf16p)
    Ww2 = par("Ww2", (D, D), bf16p)
    W1b_rep = par("W1b_rep", (128, D))
    W2bWw_rep = par("W2bWw_rep", (128, D))
    Wb_rep = par("Wb_rep", (128, D))
    iota = par("iota", (128, 128), mybir.dt.bfloat16)
    ident = par("ident", (128, 128), bf16p)
    OUT = par("OUT", (SP, D), out=True)

    eq = mybir.AluOpType.is_equal
    mult = mybir.AluOpType.mult
    addop = mybir.AluOpType.add

    with tile.TileContext(nc) as tc:
        with (
            tc.tile_pool(name="meta1", bufs=1) as metap,
            tc.tile_pool(name="gidxp", bufs=2) as gidxp,
            tc.tile_pool(name="gpool", bufs=10) as gpool,
            tc.tile_pool(name="mpool", bufs=3) as mpool,
            tc.tile_pool(name="psA", bufs=3, space="PSUM") as psA,
            tc.tile_pool(name="psT", bufs=2, space="PSUM") as psT,
            tc.tile_pool(name="small", bufs=3) as small,
            tc.tile_pool(name="stage", bufs=2) as stage,
            tc.tile_pool(name="dram", bufs=1, space="DRAM") as dram,
        ):
            # --- resident metadata ---
            def load(ap_param, shape, nm, dt=f32, pool=metap):
                t = pool.tile(list(shape), dt, name=nm, tag=nm)
                nc.scalar.dma_start(t[:], ap_param[:])
                return t

            iota_t = load(iota, (128, 128), "iota_t",
                          dt=mybir.dt.bfloat16)
            ident_t = load(ident, (128, 128), "ident_t",
                           dt=mybir.dt.bfloat16)
            w1_t = load(W1, (D, D), "w1_t", dt=mybir.dt.bfloat16)
            w2top_t = load(W2top, (D, D), "w2top_t", dt=mybir.dt.bfloat16)
            w2bot_t = load(W2bot, (D, D), "w2bot_t", dt=mybir.dt.bfloat16)
            ww1_t = load(Ww1, (D, D), "ww1_t", dt=mybir.dt.bfloat16)
            ww2_t = load(Ww2, (D, D), "ww2_t", dt=mybir.dt.bfloat16)
            w1b_t = load(W1b_rep, (128, D), "w1b_t")
            w2bww_t = load(W2bWw_rep, (128, D), "w2bww_t")
            wb_t = load(Wb_rep, (128, D), "wb_t")
            cntv_t = load(cntv, (128, G1), "cntv_t")
            cnte_t = load(cnte, (128, EG), "cnte_t")
            drel1_t = load(drel1, (128, T1 // 128), "drel1_t",
                           dt=mybir.dt.bfloat16)
            val1_t = load(val1, (128, T1 // 128), "val1_t")
            drel2_t = load(drel2, (128, T2 // 128), "drel2_t",
                           dt=mybir.dt.bfloat16)
            drel4_t = load(drel4, (128, T4 // 128), "drel4_t",
                           dt=mybir.dt.bfloat16)

            Xd_sb = metap.tile([128, G1, D], f32)    # wrapped node shard
            B_sb = metap.tile([128, G1, D], mybir.dt.bfloat16)

            bf16 = mybir.dt.bfloat16
            XdB_sb = metap.tile([128, G1, D], bf16)
            Xd_hbm = dram.tile([SP, 2 * D], bf16)
            Xe_hbm = dram.tile([EP, 2 * D], bf16)
            A_part = dram.tile([EP, D], bf16)
            A_full = dram.tile([EP, D], bf16, addr_space="Shared")
            qctr = [0]

            def sparse_step(gidx_par, gidx_cols, drel_t, val_t, srcs,
                            kcells, offs, evac):
                """srcs: (src_ap, cell_lo, cell_hi) bucket streams covering
                cells [lo, hi); slot spans from offs."""
                gidx_t = gidxp.tile([128, gidx_cols], i16, tag="gidx")
                nc.scalar.dma_start(gidx_t[:], gidx_par[:])
                tile_src = {}
                for src_ap, c_lo, c_hi in srcs:
                    base, end = offs[c_lo], offs[c_hi]
                    off = 0
                    L = end - base
                    while off < L:
                        n = min(CHUNK, L - off)
                        cols = n // 128
                        gt = gpool.tile([128, CHUNK // 128, 2 * D], bf16,
                                        tag="g")
                        nc.gpsimd.dma_gather(
                            gt[:, :cols, :], src_ap,
                            gidx_t[:, (base + off) // 16:(base + off + n) // 16],
                            n, n, 2 * D, queue_num=qctr[0] % 4)
                        qctr[0] += 1
                        if val_t is not None:
                            g2 = gpool.tile([128, CHUNK // 128, D], bf16,
                                            tag="g2")
                            vs = val_t[:, (base + off) // 128:
                                       (base + off) // 128 + cols]
                            nc.vector.tensor_mul(
                                g2[:, :cols, :], gt[:, :cols, :D],
                                vs.unsqueeze(2).broadcast_to([128, cols, D]))
                            src_t = (g2, D)
                        else:
                            src_t = (gt, 2 * D)
                        for i in range(cols):
                            tile_src[(base + off) // 128 + i] = (src_t[0], i)
                        off += n
                ntiles = offs[len(kcells)] // 128
                mb_next = 0
                m_buf = None
                for cell in range(len(kcells)):
                    kt = kcells[cell]
                    if kt == 0:
                        continue
                    cur = psA.tile([128, D], f32, tag="acc")
                    t0 = offs[cell] // 128
                    for i in range(kt):
                        t = t0 + i
                        if t >= mb_next:
                            k = min(MB, ntiles - t)
                            m_buf = mpool.tile([128, MB, 128], bf16, tag="m")
                            ib = iota_t[:].unsqueeze(1).broadcast_to(
                                [128, k, 128])
                            db = drel_t[:, t:t + k].unsqueeze(2).broadcast_to(
                                [128, k, 128])
                            nc.vector.tensor_tensor(m_buf[:, :k, :], ib, db,
                                                    eq)
                            mb_base, mb_next = t, t + k
                        gt, col = tile_src[t]
                        nc.tensor.matmul(cur[:], m_buf[:, t - mb_base, :],
                                         gt[:, col, :D],
                                         start=(i == 0), stop=(i == kt - 1))
                    evac(cell, cur)

            # ---- step 1: diffusion into Xd ----
            srcs1 = []
            for b in range(NB):
                rows = min(BUCKET, N - b * BUCKET)
                srcs1.append((Xb[b * BUCKET:b * BUCKET + rows, :],
                              b * G1, (b + 1) * G1))

            def evac1(cellidx, psum):
                b, g = divmod(cellidx, G1)
                if b == 0:
                    nc.vector.tensor_copy(Xd_sb[:, g, :], psum[:])
                else:
                    nc.vector.tensor_add(Xd_sb[:, g, :], Xd_sb[:, g, :],
                                         psum[:])

            sparse_step(gidx1, T1 // 16, drel1_t, val1_t, srcs1, KC1, OFF1, evac1)

            for g in range(G1):
                nc.vector.tensor_copy(XdB_sb[:, g, :], Xd_sb[:, g, :])
            # Xd wrapped -> row-major bf16 HBM table (step-2 gather source)
            nc.sync.dma_start(
                Xd_hbm[:, :D].rearrange("(g p) f -> p g f", p=128), XdB_sb[:])

            # ---- step 2: A[e] partials ----
            ev2 = {}

            def evac2(g, psum):
                b = g % 4
                if b == 0:
                    ev2["t"] = stage.tile([128, 4, D], bf16, tag="ev2", name="ev2t")
                    ev2["g0"] = g
                nc.vector.tensor_copy(ev2["t"][:, b, :], psum[:])
                if b == 3 or g == EG - 1:
                    nb = b + 1
                    nc.sync.dma_start(
                        A_part[ev2["g0"] * 128:(ev2["g0"] + nb) * 128, :]
                        .rearrange("(b p) f -> p b f", p=128),
                        ev2["t"][:, :nb, :])

            sparse_step(gidx2, T2 // 16, drel2_t, None,
                        [(Xd_hbm[:, :], 0, EG)], KC2, OFF2, evac2)

            # ---- AllReduce A ----
            nc.gpsimd.collective_compute(
                "AllReduce", addop,
                replica_groups=[list(range(NC))],
                ins=[A_part.opt()], outs=[A_full.opt()])

            # ---- step 3: Xe = A @ W1 + cnt_e x b1 ----
            ev3 = {}
            for g in range(EG):
                a_t = stage.tile([128, D], bf16, tag="a")
                nc.scalar.dma_start(a_t[:], A_full[g * 128:(g + 1) * 128, :])
                pT = psT.tile([D, 128], bf16, tag="t")
                nc.tensor.transpose(pT[:], a_t[:], ident_t[:])
                aT = stage.tile([D, 128], bf16, tag="aT")
                nc.vector.tensor_copy(aT[:], pT[:])
                p2 = psA.tile([128, D], f32, tag="acc")
                nc.tensor.matmul(p2[:], aT[:], w1_t[:], start=True, stop=True)
                b4 = g % 4
                if b4 == 0:
                    ev3["t"] = stage.tile([128, 4, D], bf16, tag="ev3",
                                          name="ev3t")
                    ev3["g0"] = g
                nc.vector.scalar_tensor_tensor(
                    ev3["t"][:, b4, :], w1b_t[:], cnte_t[:, g:g + 1], p2[:],
                    mult, addop)
                if b4 == 3 or g == EG - 1:
                    nb = b4 + 1
                    nc.sync.dma_start(
                        Xe_hbm[ev3["g0"] * 128:(ev3["g0"] + nb) * 128, :D]
                        .rearrange("(b p) f -> p b f", p=128),
                        ev3["t"][:, :nb, :])

            # ---- step 4: B[v] ----
            def evac4(g, psum):
                nc.vector.tensor_copy(B_sb[:, g, :], psum[:])

            sparse_step(gidx4, T4 // 16, drel4_t, None,
                        [(Xe_hbm[:, :], 0, G1)], KC4, OFF4, evac4)

            # ---- steps 5-7 ----
            for g in range(G1):
                xd = Xd_sb[:, g, :]
                cnt = cntv_t[:, g:g + 1]
                xdc = stage.tile([128, D], bf16, tag="xdc")
                nc.vector.tensor_scalar_mul(xdc[:], xd, cnt)
                pT1 = psT.tile([D, 128], bf16, tag="t")
                nc.tensor.transpose(pT1[:], xdc[:], ident_t[:])
                xdcT = stage.tile([D, 128], bf16, tag="xdcT")
                nc.vector.tensor_copy(xdcT[:], pT1[:])
                pT2 = psT.tile([D, 128], bf16, tag="t")
                nc.tensor.transpose(pT2[:], XdB_sb[:, g, :], ident_t[:])
                xdT = stage.tile([D, 128], bf16, tag="xdT")
                nc.vector.tensor_copy(xdT[:], pT2[:])
                pT3 = psT.tile([D, 128], bf16, tag="t")
                nc.tensor.transpose(pT3[:], B_sb[:, g, :], ident_t[:])
                bT = stage.tile([D, 128], bf16, tag="bT")
                nc.vector.tensor_copy(bT[:], pT3[:])
                pvT = psT.tile([D, 128], f32, tag="vt")
                nc.tensor.matmul(pvT[:], w2top_t[:], xdcT[:],
                                 start=True, stop=False)
                nc.tensor.matmul(pvT[:], w2bot_t[:], bT[:],
                                 start=False, stop=True)
                xvT = stage.tile([D, 128], bf16, tag="xvT")
                nc.vector.tensor_copy(xvT[:], pvT[:])
                po = psA.tile([128, D], f32, tag="acc")
                nc.tensor.matmul(po[:], xvT[:], ww1_t[:],
                                 start=True, stop=False)
                nc.tensor.matmul(po[:], xdT[:], ww2_t[:],
                                 start=False, stop=True)
                tmp = stage.tile([128, D], f32, tag="tmp")
                nc.vector.scalar_tensor_tensor(
                    tmp[:], w2bww_t[:], cnt, po[:], mult, addop)
                ot = stage.tile([128, D], f32, tag="ot")
                nc.vector.tensor_add(ot[:], tmp[:], wb_t[:])
                nc.sync.dma_start(OUT[g * 128:(g + 1) * 128, :], ot[:])

    nc.compile()
    return nc


def _run(inputs, n_edges, sim=False):
    meta, in_maps = _prep(inputs, n_edges)
    nc = _build(meta)
    S, SP = meta["S"], meta["SP"]
    if sim:
        from concourse import bass_interp
        ms = bass_interp.MultiCoreSim(nc, NC, require_finite=False, require_nnan=False)
        for c in range(NC):
            for k, v in in_maps[c].items():
                ms.cores[c].tensor(k)[:] = v
        ms.simulate()
        outs = [np.array(ms.cores[c].mem_tensor("OUT")).reshape(SP, D)
                for c in range(NC)]
    else:
        from concourse.bass_utils import run_bass_kernel_spmd
        res = run_bass_kernel_spmd(nc, in_maps, list(range(NC)),
                                   trace=TRACE)
        global LAST_EXEC_NS, LAST_RESULTS
        LAST_EXEC_NS = res.exec_time_ns
        LAST_RESULTS = res
        outs = [res.results[c]["OUT"] for c in range(NC)]
    return np.concatenate([o[:S] for o in outs], axis=0).astype(np.float32)


def kernel(**inputs):
    return _run(inputs, 25000, sim=False)
